# revision 5
# baseline (speedup 1.0000x reference)
"""Banded-matmul Trainium2 kernel.

Computes out = x @ (W * band_mask).T + bias for
  x: [8192, 4096] f32, W: [4096, 4096] f32, bias: [4096] f32,
  band_mask[i, j] = |i - j| <= 1024.

Strategy:
  - Data-parallel over batch across 8 NeuronCores (1024 rows each).
  - All transposes/masking folded into host-side preprocessing:
      * xT = x.T                        -> [in, batch], sharded on batch
      * W_packed = band blocks of (W*mask).T packed contiguously
      * bias_r = bias reshaped [128, 32] (partition-major per o-block)
  - On device each core computes outT_shard[o, b] = sum_j WT[j,o] xT[j,b]
    as a band-block-sparse matmul: for each 128-wide o-block only the
    j-blocks intersecting the band (|o-j| <= 1024) are loaded/multiplied.
  - Host gathers per-core outT shards and transposes back.
"""

import numpy as np

import concourse.bacc as bacc
import concourse.bass as bass
import concourse.mybir as mybir
import concourse.tile as tile
from concourse.bass_utils import run_bass_kernel_spmd

IN_F = 4096
OUT_F = 4096
BW = 1024
BATCH = 8192
N_CORES = 8
P = 128
NBLK = OUT_F // P  # 32 o-blocks / j-blocks
BBLK = BW // P  # 8: band half-width in blocks
B_LOCAL = BATCH // N_CORES  # 1024
BGRP = 512  # moving free dim per matmul
NBG = B_LOCAL // BGRP  # 2 batch groups per core

FP32 = mybir.dt.float32


def _band_range(t: int) -> tuple[int, int]:
    """Inclusive j-block range intersecting the band of o-block t."""
    return max(0, t - BBLK), min(NBLK - 1, t + BBLK)


def _band_layout():
    """Per o-block (start offset in blocks, j-block list) into W_packed."""
    offs, blocks = [], []
    off = 0
    for t in range(NBLK):
        lo, hi = _band_range(t)
        ms = list(range(lo, hi + 1))
        offs.append(off)
        blocks.append(ms)
        off += len(ms)
    return offs, blocks, off


_OFFS, _BLOCKS, _TOTAL_BLOCKS = _band_layout()


def _pack_weight(weight: np.ndarray) -> np.ndarray:
    """Pack band blocks of (W*mask).T into [128, total_blocks*128].

    Column block k (for o-block t, j-block m) holds
      W_packed[p, o_local] = W[t*128+o_local, m*128+p] * mask.
    Only the |m-t| == BBLK edge blocks need actual mask values
    (triangular); interior blocks are fully inside the band.
    """
    wt = weight.T  # [j, o] view
    r = np.arange(P)
    # j - o = 128*(m-t) + p - o_local; in band iff |j - o| <= BW
    upper = (r[:, None] <= r[None, :]).astype(np.float32)  # p <= o_local
    lower = (r[:, None] >= r[None, :]).astype(np.float32)  # p >= o_local
    cols = np.empty((P, _TOTAL_BLOCKS * P), dtype=np.float32)
    k = 0
    for t in range(NBLK):
        for m in _BLOCKS[t]:
            blk = wt[m * P : (m + 1) * P, t * P : (t + 1) * P]
            if m - t == BBLK:
                blk = blk * upper
            elif m - t == -BBLK:
                blk = blk * lower
            cols[:, k * P : (k + 1) * P] = blk
            k += 1
    return cols


def _build_program() -> bass.Bass:
    nc = bacc.Bacc("TRN2", target_bir_lowering=False, debug=False)
    xT = nc.dram_tensor("xT", [IN_F, B_LOCAL], FP32, kind="ExternalInput")
    wp = nc.dram_tensor("wp", [P, _TOTAL_BLOCKS * P], FP32, kind="ExternalInput")
    br = nc.dram_tensor("bias_r", [P, NBLK], FP32, kind="ExternalInput")
    out = nc.dram_tensor("outT", [OUT_F, B_LOCAL], FP32, kind="ExternalOutput")

    with tile.TileContext(nc) as tc:
        with (
            tc.tile_pool(name="xpool", bufs=1) as xpool,
            tc.tile_pool(name="wpool", bufs=3) as wpool,
            tc.tile_pool(name="bpool", bufs=1) as bpool,
            tc.tile_pool(name="opool", bufs=4) as opool,
            tc.tile_pool(name="pspool", bufs=8, space="PSUM") as pspool,
        ):
            btile = bpool.tile([P, NBLK], FP32, name="btile")
            nc.sync.dma_start(btile[:], br[:])

            # x kept resident in SBUF: 32 tiles x [128, 1024] = 128KB/partition
            xtiles = []
            for m in range(NBLK):
                xt = xpool.tile(
                    [P, B_LOCAL], FP32, name=f"xtile{m}", tag=f"x{m}"
                )
                nc.sync.dma_start(xt[:], xT[m * P : (m + 1) * P, :])
                xtiles.append(xt)

            for t in range(NBLK):
                ms = _BLOCKS[t]
                n_t = len(ms)
                wtile = wpool.tile([P, n_t * P], FP32, name=f"wtile{t}", tag="w")
                nc.sync.dma_start(
                    wtile[:], wp[:, _OFFS[t] * P : (_OFFS[t] + n_t) * P]
                )
                for bg in range(NBG):
                    ps = pspool.tile([P, BGRP], FP32, name=f"ps{t}_{bg}", tag="ps")
                    for ki in range(n_t):
                        nc.tensor.matmul(
                            ps[:],
                            wtile[:, ki * P : (ki + 1) * P],
                            xtiles[ms[ki]][:, bg * BGRP : (bg + 1) * BGRP],
                            start=(ki == 0),
                            stop=(ki == n_t - 1),
                        )
                    ot = opool.tile([P, BGRP], FP32, name=f"ot{t}_{bg}", tag="o")
                    nc.scalar.activation(
                        ot[:],
                        ps[:],
                        mybir.ActivationFunctionType.Identity,
                        bias=btile[:, t : t + 1],
                    )
                    nc.sync.dma_start(
                        out[t * P : (t + 1) * P, bg * BGRP : (bg + 1) * BGRP],
                        ot[:],
                    )
    nc.compile()
    return nc


_NC_CACHE = None


def _get_program() -> bass.Bass:
    global _NC_CACHE
    if _NC_CACHE is None:
        _NC_CACHE = _build_program()
    return _NC_CACHE


def _run(x: np.ndarray, weight: np.ndarray, bias: np.ndarray, trace: bool = False):
    x = np.ascontiguousarray(np.asarray(x, dtype=np.float32))
    weight = np.ascontiguousarray(np.asarray(weight, dtype=np.float32))
    bias = np.ascontiguousarray(np.asarray(bias, dtype=np.float32))

    xT = np.ascontiguousarray(x.T)  # [in, batch]
    wp = _pack_weight(weight)
    br = np.ascontiguousarray(bias.reshape(NBLK, P).T)  # [128, 32]

    in_maps = []
    for c in range(N_CORES):
        shard = np.ascontiguousarray(xT[:, c * B_LOCAL : (c + 1) * B_LOCAL])
        in_maps.append({"xT": shard, "wp": wp, "bias_r": br})

    nc = _get_program()
    res = run_bass_kernel_spmd(nc, in_maps, list(range(N_CORES)), trace=trace)
    outT = np.concatenate([res.results[c]["outT"] for c in range(N_CORES)], axis=1)
    out = np.ascontiguousarray(outT.T)  # [batch, out]
    return out, res


def kernel(x: np.ndarray, weight: np.ndarray, bias: np.ndarray) -> np.ndarray:
    out, _ = _run(x, weight, bias, trace=False)
    return out


# revision 9
# speedup vs baseline: 3.1639x; 3.1639x over previous
"""Banded-matmul Trainium2 kernel.

Computes out = x @ (W * band_mask).T + bias for
  x: [8192, 4096] f32, W: [4096, 4096] f32, bias: [4096] f32,
  band_mask[i, j] = |i - j| <= 1024.

Strategy:
  - Data-parallel over batch across 8 NeuronCores (1024 rows each).
  - All transposes/masking folded into host-side preprocessing:
      * xT = x.T                        -> [in, batch], sharded on batch
      * W_packed = band blocks of (W*mask).T packed contiguously
      * bias_r = bias reshaped [128, 32] (partition-major per o-block)
  - On device each core computes outT_shard[o, b] = sum_j WT[j,o] xT[j,b]
    as a band-block-sparse matmul: for each 128-wide o-block only the
    j-blocks intersecting the band (|o-j| <= 1024) are loaded/multiplied.
  - Host gathers per-core outT shards and transposes back.
"""

import numpy as np

import concourse.bacc as bacc
import concourse.bass as bass
import concourse.mybir as mybir
import concourse.tile as tile
from concourse.bass_utils import run_bass_kernel_spmd

IN_F = 4096
OUT_F = 4096
BW = 1024
BATCH = 8192
N_CORES = 8
P = 128
NBLK = OUT_F // P  # 32 o-blocks / j-blocks
BBLK = BW // P  # 8: band half-width in blocks
B_LOCAL = BATCH // N_CORES  # 1024
BGRP = 512  # moving free dim per matmul
NBG = B_LOCAL // BGRP  # 2 batch groups per core

FP32 = mybir.dt.float32
FP32R = mybir.dt.float32r  # TF32-like PE mode: 1 cycle/row (fp32: 4)


def _band_range(t: int) -> tuple[int, int]:
    """Inclusive j-block range intersecting the band of o-block t."""
    return max(0, t - BBLK), min(NBLK - 1, t + BBLK)


def _band_layout():
    """Per o-block (start offset in blocks, j-block list) into W_packed."""
    offs, blocks = [], []
    off = 0
    for t in range(NBLK):
        lo, hi = _band_range(t)
        ms = list(range(lo, hi + 1))
        offs.append(off)
        blocks.append(ms)
        off += len(ms)
    return offs, blocks, off


_OFFS, _BLOCKS, _TOTAL_BLOCKS = _band_layout()


def _pack_weight(weight: np.ndarray) -> np.ndarray:
    """Pack band blocks of (W*mask).T into [128, total_blocks*128].

    Column block k (for o-block t, j-block m) holds
      W_packed[p, o_local] = W[t*128+o_local, m*128+p] * mask.
    Only the |m-t| == BBLK edge blocks need actual mask values
    (triangular); interior blocks are fully inside the band.
    """
    wt = weight.T  # [j, o] view
    r = np.arange(P)
    # j - o = 128*(m-t) + p - o_local; in band iff |j - o| <= BW
    upper = (r[:, None] <= r[None, :]).astype(np.float32)  # p <= o_local
    lower = (r[:, None] >= r[None, :]).astype(np.float32)  # p >= o_local
    cols = np.empty((P, _TOTAL_BLOCKS * P), dtype=np.float32)
    k = 0
    for t in range(NBLK):
        for m in _BLOCKS[t]:
            blk = wt[m * P : (m + 1) * P, t * P : (t + 1) * P]
            if m - t == BBLK:
                blk = blk * upper
            elif m - t == -BBLK:
                blk = blk * lower
            cols[:, k * P : (k + 1) * P] = blk
            k += 1
    return cols


def _build_program() -> bass.Bass:
    nc = bacc.Bacc("TRN2", target_bir_lowering=False, debug=False)
    xT = nc.dram_tensor("xT", [IN_F, B_LOCAL], FP32R, kind="ExternalInput")
    wp = nc.dram_tensor("wp", [P, _TOTAL_BLOCKS * P], FP32R, kind="ExternalInput")
    br = nc.dram_tensor("bias_r", [P, NBLK], FP32, kind="ExternalInput")
    out = nc.dram_tensor("outT", [OUT_F, B_LOCAL], FP32, kind="ExternalOutput")

    with tile.TileContext(nc) as tc:
        with (
            tc.tile_pool(name="xpool", bufs=1) as xpool,
            tc.tile_pool(name="wpool", bufs=3) as wpool,
            tc.tile_pool(name="bpool", bufs=1) as bpool,
            tc.tile_pool(name="opool", bufs=4) as opool,
            tc.tile_pool(name="pspool", bufs=8, space="PSUM") as pspool,
        ):
            btile = bpool.tile([P, NBLK], FP32, name="btile")
            nc.sync.dma_start(btile[:], br[:])

            # x kept resident in SBUF: 32 tiles x [128, 1024] = 128KB/partition
            xtiles = []
            for m in range(NBLK):
                xt = xpool.tile(
                    [P, B_LOCAL], FP32R, name=f"xtile{m}", tag=f"x{m}"
                )
                nc.sync.dma_start(xt[:], xT[m * P : (m + 1) * P, :])
                xtiles.append(xt)

            for t in range(NBLK):
                ms = _BLOCKS[t]
                n_t = len(ms)
                wtile = wpool.tile([P, n_t * P], FP32R, name=f"wtile{t}", tag="w")
                nc.sync.dma_start(
                    wtile[:], wp[:, _OFFS[t] * P : (_OFFS[t] + n_t) * P]
                )
                for bg in range(NBG):
                    ps = pspool.tile([P, BGRP], FP32, name=f"ps{t}_{bg}", tag="ps")
                    for ki in range(n_t):
                        nc.tensor.matmul(
                            ps[:],
                            wtile[:, ki * P : (ki + 1) * P],
                            xtiles[ms[ki]][:, bg * BGRP : (bg + 1) * BGRP],
                            start=(ki == 0),
                            stop=(ki == n_t - 1),
                        )
                    ot = opool.tile([P, BGRP], FP32, name=f"ot{t}_{bg}", tag="o")
                    nc.scalar.activation(
                        ot[:],
                        ps[:],
                        mybir.ActivationFunctionType.Identity,
                        bias=btile[:, t : t + 1],
                    )
                    nc.sync.dma_start(
                        out[t * P : (t + 1) * P, bg * BGRP : (bg + 1) * BGRP],
                        ot[:],
                    )
    nc.compile()
    return nc


_NC_CACHE = None


def _get_program() -> bass.Bass:
    global _NC_CACHE
    if _NC_CACHE is None:
        _NC_CACHE = _build_program()
    return _NC_CACHE


def _run(x: np.ndarray, weight: np.ndarray, bias: np.ndarray, trace: bool = False):
    x = np.ascontiguousarray(np.asarray(x, dtype=np.float32))
    weight = np.ascontiguousarray(np.asarray(weight, dtype=np.float32))
    bias = np.ascontiguousarray(np.asarray(bias, dtype=np.float32))

    xT = np.ascontiguousarray(x.T)  # [in, batch]
    wp = _pack_weight(weight)
    br = np.ascontiguousarray(bias.reshape(NBLK, P).T)  # [128, 32]

    in_maps = []
    for c in range(N_CORES):
        shard = np.ascontiguousarray(xT[:, c * B_LOCAL : (c + 1) * B_LOCAL])
        in_maps.append({"xT": shard, "wp": wp, "bias_r": br})

    nc = _get_program()
    res = run_bass_kernel_spmd(nc, in_maps, list(range(N_CORES)), trace=trace)
    outT = np.concatenate([res.results[c]["outT"] for c in range(N_CORES)], axis=1)
    out = np.ascontiguousarray(outT.T)  # [batch, out]
    return out, res


def kernel(x: np.ndarray, weight: np.ndarray, bias: np.ndarray) -> np.ndarray:
    out, _ = _run(x, weight, bias, trace=False)
    return out


# revision 10
# speedup vs baseline: 3.4689x; 1.0964x over previous
"""Banded-matmul Trainium2 kernel.

Computes out = x @ (W * band_mask).T + bias for
  x: [8192, 4096] f32, W: [4096, 4096] f32, bias: [4096] f32,
  band_mask[i, j] = |i - j| <= 1024.

Strategy:
  - Data-parallel over batch across 8 NeuronCores (1024 rows each).
  - All transposes/masking folded into host-side preprocessing:
      * xT = x.T                        -> [in, batch], sharded on batch
      * W_packed = band blocks of (W*mask).T packed contiguously
      * bias_r = bias reshaped [128, 32] (partition-major per o-block)
  - On device each core computes outT_shard[o, b] = sum_j WT[j,o] xT[j,b]
    as a band-block-sparse matmul: for each 128-wide o-block only the
    j-blocks intersecting the band (|o-j| <= 1024) are loaded/multiplied.
  - Host gathers per-core outT shards and transposes back.
"""

import numpy as np

import concourse.bacc as bacc
import concourse.bass as bass
import concourse.mybir as mybir
import concourse.tile as tile
from concourse.bass_utils import run_bass_kernel_spmd

IN_F = 4096
OUT_F = 4096
BW = 1024
BATCH = 8192
N_CORES = 8
P = 128
NBLK = OUT_F // P  # 32 o-blocks / j-blocks
BBLK = BW // P  # 8: band half-width in blocks
B_LOCAL = BATCH // N_CORES  # 1024
BGRP = 512  # moving free dim per matmul
NBG = B_LOCAL // BGRP  # 2 batch groups per core

FP32 = mybir.dt.float32
FP32R = mybir.dt.float32r  # TF32-like PE mode: 1 cycle/row (fp32: 4)


def _band_range(t: int) -> tuple[int, int]:
    """Inclusive j-block range intersecting the band of o-block t."""
    return max(0, t - BBLK), min(NBLK - 1, t + BBLK)


def _band_layout():
    """Per o-block (start offset in blocks, j-block list) into W_packed."""
    offs, blocks = [], []
    off = 0
    for t in range(NBLK):
        lo, hi = _band_range(t)
        ms = list(range(lo, hi + 1))
        offs.append(off)
        blocks.append(ms)
        off += len(ms)
    return offs, blocks, off


_OFFS, _BLOCKS, _TOTAL_BLOCKS = _band_layout()


def _pack_weight(weight: np.ndarray) -> np.ndarray:
    """Pack band blocks of (W*mask).T into [128, total_blocks*128].

    Column block k (for o-block t, j-block m) holds
      W_packed[p, o_local] = W[t*128+o_local, m*128+p] * mask.
    Only the |m-t| == BBLK edge blocks need actual mask values
    (triangular); interior blocks are fully inside the band.
    """
    wt = weight.T  # [j, o] view
    r = np.arange(P)
    # j - o = 128*(m-t) + p - o_local; in band iff |j - o| <= BW
    upper = (r[:, None] <= r[None, :]).astype(np.float32)  # p <= o_local
    lower = (r[:, None] >= r[None, :]).astype(np.float32)  # p >= o_local
    cols = np.empty((P, _TOTAL_BLOCKS * P), dtype=np.float32)
    k = 0
    for t in range(NBLK):
        for m in _BLOCKS[t]:
            blk = wt[m * P : (m + 1) * P, t * P : (t + 1) * P]
            if m - t == BBLK:
                blk = blk * upper
            elif m - t == -BBLK:
                blk = blk * lower
            cols[:, k * P : (k + 1) * P] = blk
            k += 1
    return cols


def _build_program() -> bass.Bass:
    nc = bacc.Bacc("TRN2", target_bir_lowering=False, debug=False)
    xT = nc.dram_tensor("xT", [IN_F, B_LOCAL], FP32R, kind="ExternalInput")
    wp = nc.dram_tensor("wp", [P, _TOTAL_BLOCKS * P], FP32R, kind="ExternalInput")
    br = nc.dram_tensor("bias_r", [P, NBLK], FP32, kind="ExternalInput")
    out = nc.dram_tensor("outT", [OUT_F, B_LOCAL], FP32, kind="ExternalOutput")

    with tile.TileContext(nc) as tc:
        with (
            tc.tile_pool(name="xpool", bufs=1) as xpool,
            tc.tile_pool(name="wpool", bufs=3) as wpool,
            tc.tile_pool(name="bpool", bufs=1) as bpool,
            tc.tile_pool(name="opool", bufs=4) as opool,
            tc.tile_pool(name="pspool", bufs=8, space="PSUM") as pspool,
        ):
            btile = bpool.tile([P, NBLK], FP32, name="btile")
            nc.sync.dma_start(btile[:], br[:])

            # x resident in SBUF as 64 half-tiles [128, 512]; loaded lazily
            # in band order so the first matmuls start after ~5MB of DMA
            # instead of the full 16MB x preload.
            xh = [[None, None] for _ in range(NBLK)]

            def load_x(m):
                for bg in range(NBG):
                    xt = xpool.tile(
                        [P, BGRP], FP32R, name=f"x{m}_{bg}", tag=f"x{m}_{bg}"
                    )
                    nc.sync.dma_start(
                        xt[:],
                        xT[m * P : (m + 1) * P, bg * BGRP : (bg + 1) * BGRP],
                    )
                    xh[m][bg] = xt

            for t in range(NBLK):
                ms = _BLOCKS[t]
                n_t = len(ms)
                wtile = wpool.tile([P, n_t * P], FP32R, name=f"wtile{t}", tag="w")
                nc.sync.dma_start(
                    wtile[:], wp[:, _OFFS[t] * P : (_OFFS[t] + n_t) * P]
                )
                for m in ms:
                    if xh[m][0] is None:
                        load_x(m)
                ps = [
                    pspool.tile([P, BGRP], FP32, name=f"ps{t}_{bg}", tag="ps")
                    for bg in range(NBG)
                ]
                for ki in range(n_t):
                    wslice = wtile[:, ki * P : (ki + 1) * P]
                    for bg in range(NBG):
                        nc.tensor.matmul(
                            ps[bg][:],
                            wslice,
                            xh[ms[ki]][bg][:],
                            start=(ki == 0),
                            stop=(ki == n_t - 1),
                            skip_group_check=True,
                        )
                for bg in range(NBG):
                    ot = opool.tile([P, BGRP], FP32, name=f"ot{t}_{bg}", tag="o")
                    nc.scalar.activation(
                        ot[:],
                        ps[bg][:],
                        mybir.ActivationFunctionType.Identity,
                        bias=btile[:, t : t + 1],
                    )
                    nc.sync.dma_start(
                        out[t * P : (t + 1) * P, bg * BGRP : (bg + 1) * BGRP],
                        ot[:],
                    )
    nc.compile()
    return nc


_NC_CACHE = None


def _get_program() -> bass.Bass:
    global _NC_CACHE
    if _NC_CACHE is None:
        _NC_CACHE = _build_program()
    return _NC_CACHE


def _run(x: np.ndarray, weight: np.ndarray, bias: np.ndarray, trace: bool = False):
    x = np.ascontiguousarray(np.asarray(x, dtype=np.float32))
    weight = np.ascontiguousarray(np.asarray(weight, dtype=np.float32))
    bias = np.ascontiguousarray(np.asarray(bias, dtype=np.float32))

    xT = np.ascontiguousarray(x.T)  # [in, batch]
    wp = _pack_weight(weight)
    br = np.ascontiguousarray(bias.reshape(NBLK, P).T)  # [128, 32]

    in_maps = []
    for c in range(N_CORES):
        shard = np.ascontiguousarray(xT[:, c * B_LOCAL : (c + 1) * B_LOCAL])
        in_maps.append({"xT": shard, "wp": wp, "bias_r": br})

    nc = _get_program()
    res = run_bass_kernel_spmd(nc, in_maps, list(range(N_CORES)), trace=trace)
    outT = np.concatenate([res.results[c]["outT"] for c in range(N_CORES)], axis=1)
    out = np.ascontiguousarray(outT.T)  # [batch, out]
    return out, res


def kernel(x: np.ndarray, weight: np.ndarray, bias: np.ndarray) -> np.ndarray:
    out, _ = _run(x, weight, bias, trace=False)
    return out


# revision 15
# speedup vs baseline: 3.5308x; 1.0178x over previous
"""Banded-matmul Trainium2 kernel.

Computes out = x @ (W * band_mask).T + bias for
  x: [8192, 4096] f32, W: [4096, 4096] f32, bias: [4096] f32,
  band_mask[i, j] = |i - j| <= 1024.

Strategy:
  - Data-parallel over batch across 8 NeuronCores (1024 rows each).
  - All transposes/masking folded into host-side preprocessing:
      * xT = x.T                        -> [in, batch], sharded on batch
      * W_packed = band blocks of (W*mask).T packed contiguously
      * bias_r = bias reshaped [128, 32] (partition-major per o-block)
  - On device each core computes outT_shard[o, b] = sum_j WT[j,o] xT[j,b]
    as a band-block-sparse matmul: for each 128-wide o-block only the
    j-blocks intersecting the band (|o-j| <= 1024) are loaded/multiplied.
  - Host gathers per-core outT shards and transposes back.
"""

import numpy as np

import concourse.bacc as bacc
import concourse.bass as bass
import concourse.mybir as mybir
import concourse.tile as tile
from concourse.bass_utils import run_bass_kernel_spmd

IN_F = 4096
OUT_F = 4096
BW = 1024
BATCH = 8192
N_CORES = 8
P = 128
NBLK = OUT_F // P  # 32 o-blocks / j-blocks
BBLK = BW // P  # 8: band half-width in blocks
B_LOCAL = BATCH // N_CORES  # 1024
BGRP = 512  # moving free dim per matmul
NBG = B_LOCAL // BGRP  # 2 batch groups per core

FP32 = mybir.dt.float32
FP32R = mybir.dt.float32r  # TF32-like PE mode: 1 cycle/row (fp32: 4)


def _band_range(t: int) -> tuple[int, int]:
    """Inclusive j-block range intersecting the band of o-block t."""
    return max(0, t - BBLK), min(NBLK - 1, t + BBLK)


def _band_layout():
    """Per o-block (start offset in blocks, j-block list) into W_packed."""
    offs, blocks = [], []
    off = 0
    for t in range(NBLK):
        lo, hi = _band_range(t)
        ms = list(range(lo, hi + 1))
        offs.append(off)
        blocks.append(ms)
        off += len(ms)
    return offs, blocks, off


_OFFS, _BLOCKS, _TOTAL_BLOCKS = _band_layout()


def _pack_weight(weight: np.ndarray) -> np.ndarray:
    """Pack band blocks of (W*mask).T into [128, total_blocks*128].

    Column block k (for o-block t, j-block m) holds
      W_packed[p, o_local] = W[t*128+o_local, m*128+p] * mask.
    Only the |m-t| == BBLK edge blocks need actual mask values
    (triangular); interior blocks are fully inside the band.
    """
    wt = weight.T  # [j, o] view
    r = np.arange(P)
    # j - o = 128*(m-t) + p - o_local; in band iff |j - o| <= BW
    upper = (r[:, None] <= r[None, :]).astype(np.float32)  # p <= o_local
    lower = (r[:, None] >= r[None, :]).astype(np.float32)  # p >= o_local
    cols = np.empty((P, _TOTAL_BLOCKS * P), dtype=np.float32)
    k = 0
    for t in range(NBLK):
        for m in _BLOCKS[t]:
            blk = wt[m * P : (m + 1) * P, t * P : (t + 1) * P]
            if m - t == BBLK:
                blk = blk * upper
            elif m - t == -BBLK:
                blk = blk * lower
            cols[:, k * P : (k + 1) * P] = blk
            k += 1
    return cols


def _build_program() -> bass.Bass:
    nc = bacc.Bacc("TRN2", target_bir_lowering=False, debug=False)
    xT = nc.dram_tensor("xT", [IN_F, B_LOCAL], FP32R, kind="ExternalInput")
    wp = nc.dram_tensor("wp", [P, _TOTAL_BLOCKS * P], FP32R, kind="ExternalInput")
    br = nc.dram_tensor("bias_r", [P, NBLK], FP32, kind="ExternalInput")
    out = nc.dram_tensor("outT", [OUT_F, B_LOCAL], FP32, kind="ExternalOutput")

    with tile.TileContext(nc) as tc:
        with (
            tc.tile_pool(name="xpool", bufs=1) as xpool,
            tc.tile_pool(name="wpool", bufs=4) as wpool,
            tc.tile_pool(name="bpool", bufs=1) as bpool,
            tc.tile_pool(name="opool", bufs=4) as opool,
            tc.tile_pool(name="pspool", bufs=8, space="PSUM") as pspool,
        ):
            btile = bpool.tile([P, NBLK], FP32, name="btile")
            nc.sync.dma_start(btile[:], br[:])

            # x resident in SBUF as 64 half-tiles [128, 512]; loaded lazily
            # in band order so the first matmuls start after ~5MB of DMA
            # instead of the full 16MB x preload.
            xh = [[None, None] for _ in range(NBLK)]

            def load_x(m):
                for bg in range(NBG):
                    xt = xpool.tile(
                        [P, BGRP], FP32R, name=f"x{m}_{bg}", tag=f"x{m}_{bg}"
                    )
                    nc.sync.dma_start(
                        xt[:],
                        xT[m * P : (m + 1) * P, bg * BGRP : (bg + 1) * BGRP],
                    )
                    xh[m][bg] = xt

            for t in range(NBLK):
                ms = _BLOCKS[t]
                n_t = len(ms)
                if t == 0:
                    # Split the first W slab so the first matmul only waits
                    # on a 128KB chunk instead of the full 1.1MB slab.
                    wa = wpool.tile([P, 2 * P], FP32R, name="w0a", tag="w0a")
                    nc.sync.dma_start(wa[:], wp[:, 0 : 2 * P])
                    wb = wpool.tile(
                        [P, (n_t - 2) * P], FP32R, name="w0b", tag="w0b"
                    )
                    nc.sync.dma_start(wb[:], wp[:, 2 * P : n_t * P])

                    def wsl(ki, n_t=n_t, wa=wa, wb=wb):
                        if ki < 2:
                            return wa[:, ki * P : (ki + 1) * P]
                        return wb[:, (ki - 2) * P : (ki - 1) * P]
                else:
                    wtile = wpool.tile(
                        [P, n_t * P], FP32R, name=f"wtile{t}", tag="w"
                    )
                    nc.sync.dma_start(
                        wtile[:], wp[:, _OFFS[t] * P : (_OFFS[t] + n_t) * P]
                    )

                    def wsl(ki, wtile=wtile):
                        return wtile[:, ki * P : (ki + 1) * P]

                for m in ms:
                    if xh[m][0] is None:
                        load_x(m)
                ps = [
                    pspool.tile([P, BGRP], FP32, name=f"ps{t}_{bg}", tag="ps")
                    for bg in range(NBG)
                ]
                for ki in range(n_t):
                    wslice = wsl(ki)
                    for bg in range(NBG):
                        nc.tensor.matmul(
                            ps[bg][:],
                            wslice,
                            xh[ms[ki]][bg][:],
                            start=(ki == 0),
                            stop=(ki == n_t - 1),
                            skip_group_check=True,
                        )
                for bg in range(NBG):
                    ot = opool.tile([P, BGRP], FP32, name=f"ot{t}_{bg}", tag="o")
                    nc.scalar.activation(
                        ot[:],
                        ps[bg][:],
                        mybir.ActivationFunctionType.Identity,
                        bias=btile[:, t : t + 1],
                    )
                    nc.scalar.dma_start(
                        out[t * P : (t + 1) * P, bg * BGRP : (bg + 1) * BGRP],
                        ot[:],
                    )
    nc.compile()
    return nc


_NC_CACHE = None


def _get_program() -> bass.Bass:
    global _NC_CACHE
    if _NC_CACHE is None:
        _NC_CACHE = _build_program()
    return _NC_CACHE


def _run(x: np.ndarray, weight: np.ndarray, bias: np.ndarray, trace: bool = False):
    x = np.ascontiguousarray(np.asarray(x, dtype=np.float32))
    weight = np.ascontiguousarray(np.asarray(weight, dtype=np.float32))
    bias = np.ascontiguousarray(np.asarray(bias, dtype=np.float32))

    xT = np.ascontiguousarray(x.T)  # [in, batch]
    wp = _pack_weight(weight)
    br = np.ascontiguousarray(bias.reshape(NBLK, P).T)  # [128, 32]

    in_maps = []
    for c in range(N_CORES):
        shard = np.ascontiguousarray(xT[:, c * B_LOCAL : (c + 1) * B_LOCAL])
        in_maps.append({"xT": shard, "wp": wp, "bias_r": br})

    nc = _get_program()
    res = run_bass_kernel_spmd(nc, in_maps, list(range(N_CORES)), trace=trace)
    outT = np.concatenate([res.results[c]["outT"] for c in range(N_CORES)], axis=1)
    out = np.ascontiguousarray(outT.T)  # [batch, out]
    return out, res


def kernel(x: np.ndarray, weight: np.ndarray, bias: np.ndarray) -> np.ndarray:
    out, _ = _run(x, weight, bias, trace=False)
    return out
